# revision 32
# baseline (speedup 1.0000x reference)
"""Block-sparse linear y = x @ W^T on 8 Trainium2 NeuronCores.

The 32x32 block structure (50% block density, random scatter) is not
exploitable on the 128x128 PE array (matmul cost is moving-rows only, and
neither 128-aligned skip opportunities nor packable col-quadruples occur at
this density), so W^T is densified on the host (cheap scatter-add) and run
as a dense GEMM sharded 4-way over tokens x 2-way over out_features
(8 cores, no collectives).

All operands are bf16: the PE runs bf16 at the same 1 cycle/row as
float32r, but DMA traffic halves (x 4MB + W 4MB + y 2MB per core), which
puts DMA (~28us) far under the PE stream (~59us = 256 matmuls x 512 rows
@ ~2.22GHz) and makes the schedule trivially overlappable. bf16 rounding
of inputs + outputs keeps rel err ~1e-3, well inside the 2e-2 gate.

Schedule per core, two phases with no psum partials:
 (1) out-half n=0, k-outer m-inner: each k-step's x^T k-tile and W tile
     arrive PACKED as one DMA chunk (xw layout below); 8 psum banks
     accumulate the full K; drains (DVE fp32->bf16 casts + 2 batched y
     DMAs) overlap phase 2.
 (2) out-half n=1, m-outer k-inner: each bank runs its full 16-k chain
     then drains immediately; bank 7 runs as two 256-col half-chains in
     separate psum banks so only half a bank's cast + 64KB DMA trail the
     final matmul.
14 dummy 256-row warmup matmuls ramp the PE clock gate during the DMA
head so real matmuls run at ~full clock from the start (an idle gap
resets the ~3us busy-ramp, so warmups must bridge the data gate; more
warmups than the gate needs just delays the stream).

Measured HW model (from perfetto traces of this kernel):
 - each dma_start costs ~650ns of DIRECT2D descriptor generation,
   serially on its issuing sequencer -> issues are spread across the
   three DMA-capable queues (sync/scalar/gpsimd), zipped in strict
   need-order because the rings drain roughly in issue order;
 - a chunk's transfer costs ~max(1us fixed, bytes/~300GB/s shared HBM)
   regardless of partition-split tricks, so packing x+W per k-tile into
   ONE chunk (10 input dma_starts total) halves the fixed latencies and
   semaphore hops on the critical early stream;
 - fixed overheads inside the measured window: ~1.1us engine start and
   an ~8.4us NRT teardown (57 EVENT_SEMAPHORE sweeps per engine,
   PE-sequencer-bound at ~115ns each) that no kernel structure avoids;
 - PE steady state: 512-row bf16 matmul = 220ns (~2.33GHz), LDWEIGHTS
   ~97ns fully hidden; stream floor 256 x 220ns = 56.3us.
Exec ~74-78us depending on machine state (baseline fp32r kernel: ~78.6us);
run-to-run jitter is +-1.5-2us (8 cores share HBM), so schedule changes
need multi-run A/B to evaluate.
"""

import numpy as np
import ml_dtypes

TOKENS, IN_F, OUT_F = 4096, 2048, 2048
BLOCK = 32
N_CORES = 8
TG, OG = 4, 2  # token groups x out-feature groups
T_SH = TOKENS // TG  # 1024 tokens per core
O_SH = OUT_F // OG  # 1024 out features per core
P = 128
NFREE = 512  # PSUM bank free dim (fp32)
KT = IN_F // P  # 16 k tiles
MT = T_SH // P  # 8 psum banks

N_WARM = 14  # dummy 256-row matmuls to ramp the PE clock during the DMA head

MM_DTYPE = "bfloat16"  # informational; kernel is bf16-only
TRACE = False  # set by test.py to capture an NTFF profile

_nc_cache = {}
_last_result = None  # BassKernelResults of the most recent run (for test.py)


def _build_nc():
    import concourse.mybir as mybir
    import concourse.tile as tile
    from concourse import bacc

    if "nc" in _nc_cache:
        return _nc_cache["nc"]

    bf16 = mybir.dt.bfloat16
    f32 = mybir.dt.float32

    nc = bacc.Bacc(None, target_bir_lowering=False)
    # Host-pre-blocked inputs (exact SBUF layouts; all DMAs are linear):
    # xw: per k-tile, x^T columns and the phase-1 W tile PACKED side by
    #     side: [P][KT][T_SH + NFREE] (cols 0:1024 = x^T k-tile, cols
    #     1024:1536 = W^T n=0 k-tile). One dma_start per k-span delivers
    #     everything that k-step needs — per-chunk fixed latency (~1us,
    #     descriptor/completion-bound) and semaphore hops are paid once
    #     instead of twice in the critical early stream.
    # w1: phase-2 W tiles [P][KT][NFREE].
    # y:  [2][P][MT][NFREE] bf16; host reassembles tokens/outs.
    XWF = T_SH + NFREE
    xw_d = nc.dram_tensor("xw", [P, KT, XWF], bf16, kind="ExternalInput")
    w1_d = nc.dram_tensor("w1", [P, KT, NFREE], bf16, kind="ExternalInput")
    y_d = nc.dram_tensor("y", [2, P, MT, NFREE], bf16, kind="ExternalOutput")

    with tile.TileContext(nc) as tc:
        with (
            tc.tile_pool(name="xp", bufs=1) as xp,
            tc.tile_pool(name="wp", bufs=1) as wp,
            tc.tile_pool(name="op", bufs=1) as op,
            tc.tile_pool(name="ps", bufs=1, space="PSUM") as ps,
        ):
            # Warm the PE clock gate with dummy matmuls while the first DMA
            # chunks land (~2us): PE busy-time ramps the HAM clock so real
            # matmuls run at full rate almost immediately.
            zt = xp.tile([P, 256], bf16, tag="warm", name="warm")
            nc.gpsimd.memset(zt[:], 0.0)
            warm_ps = ps.tile([P, NFREE], f32, tag="ps0", name="warm_ps")
            for _ in range(N_WARM):
                nc.tensor.matmul(
                    warm_ps[:, 0:256], zt[:, 0:P], zt[:], start=True, stop=True
                )

            xw = xp.tile([P, KT, XWF], bf16, tag="xw", name="xw")
            w1t = wp.tile([P, KT, NFREE], bf16, tag="w1", name="w1")
            ot = [
                op.tile([P, MT, NFREE], bf16, tag=f"ot{n}", name=f"ot{n}")
                for n in range(2)
            ]

            def dma_xw(eng, k0, k1):
                eng.dma_start(xw[:, k0:k1, :], xw_d[:, k0:k1, :])

            def dma_w1(eng, k0, k1):
                eng.dma_start(w1t[:, k0:k1, :], w1_d[:, k0:k1, :])

            # DMA model learned from traces: each dma_start costs ~650ns of
            # DIRECT2D descriptor generation serially on its issuing
            # sequencer, and a chunk's transfer costs roughly
            # max(~1us fixed, bytes/~300GB/s shared HBM). Chunks are zipped
            # in strict need-order round-robin across the three DMA-capable
            # sequencers (the rings drain roughly in issue order, so a
            # late-need chunk issued early would starve the PE of
            # earlier-needed data). Small k-spans first for a quick gate,
            # larger spans later.
            dma_xw(nc.sync, 0, 1)       # slot 0
            dma_xw(nc.scalar, 1, 2)
            dma_xw(nc.gpsimd, 2, 4)
            dma_xw(nc.sync, 4, 6)       # slot 1
            dma_xw(nc.scalar, 6, 8)
            dma_xw(nc.gpsimd, 8, 10)
            dma_xw(nc.sync, 10, 12)     # slot 2
            dma_xw(nc.scalar, 12, 16)
            dma_w1(nc.gpsimd, 0, 8)
            dma_w1(nc.sync, 8, 16)      # slot 3

            psums = [
                ps.tile([P, NFREE], f32, tag=f"ps{m}", name=f"ps{m}")
                for m in range(MT)
            ]

            # ---- Phase 1: n=0, k-outer m-inner (matches the DMA stream);
            # full-K accumulation in 8 psum banks, no partials. ----
            for k in range(KT):
                for m in range(MT):
                    nc.tensor.matmul(
                        psums[m][:],
                        xw[:, k, m * P : (m + 1) * P],
                        xw[:, k, T_SH : T_SH + NFREE],
                        start=(k == 0),
                        stop=(k == KT - 1),
                    )
            for m in range(MT):
                nc.vector.tensor_copy(ot[0][:, m, :], psums[m][:])
            nc.scalar.dma_start(y_d[0, :, 0:4, :], ot[0][:, 0:4, :])
            nc.scalar.dma_start(y_d[0, :, 4:8, :], ot[0][:, 4:8, :])

            # ---- Phase 2: n=1, m-outer k-inner so each bank drains the
            # moment its chain finishes. Bank 7 runs as two 256-col
            # half-chains so only a half-bank cast + 64KB DMA trail the
            # final matmul. ----
            for m in range(MT):
                if m < MT - 1:
                    for k in range(KT):
                        nc.tensor.matmul(
                            psums[m][:],
                            xw[:, k, m * P : (m + 1) * P],
                            w1t[:, k, :],
                            start=(k == 0),
                            stop=(k == KT - 1),
                        )
                    nc.vector.tensor_copy(ot[1][:, m, :], psums[m][:])
                else:
                    # Two 256-col half-chains in DIFFERENT psum banks (a
                    # shared bank would serialize chain B's start=True
                    # against chain A's cast). Half A's cast+DMA overlap
                    # chain B, so only half a bank's drain trails the
                    # final matmul.
                    for h, pbank in ((0, psums[m]), (1, psums[0])):
                        c0, c1 = h * 256, (h + 1) * 256
                        for k in range(KT):
                            nc.tensor.matmul(
                                pbank[:, 0:256],
                                xw[:, k, m * P : (m + 1) * P],
                                w1t[:, k, c0:c1],
                                start=(k == 0),
                                stop=(k == KT - 1),
                            )
                        nc.vector.tensor_copy(
                            ot[1][:, m, c0:c1], pbank[:, 0:256]
                        )
                        nc.sync.dma_start(
                            y_d[1, :, m, c0:c1], ot[1][:, m, c0:c1]
                        )
                if m == 3:
                    nc.scalar.dma_start(y_d[1, :, 0:4, :], ot[1][:, 0:4, :])
                elif m == 6:
                    nc.scalar.dma_start(y_d[1, :, 4:7, :], ot[1][:, 4:7, :])

    nc.compile()
    _nc_cache["nc"] = nc
    return nc


def _densify_wT(weight_blocks, block_rows, block_cols):
    """Scatter-add the 32x32 blocks into dense W^T [in_features, out_features]."""
    nc_blk = IN_F // BLOCK
    nr_blk = OUT_F // BLOCK
    wcr = np.zeros((nc_blk, nr_blk, BLOCK, BLOCK), np.float32)
    # block b occupies W[32r:32r+32, 32c:32c+32]; W^T gets the transposed block
    np.add.at(
        wcr,
        (block_cols.astype(np.int64), block_rows.astype(np.int64)),
        np.swapaxes(weight_blocks.astype(np.float32, copy=False), 1, 2),
    )
    return np.ascontiguousarray(wcr.transpose(0, 2, 1, 3).reshape(IN_F, OUT_F))


def _pack_core_inputs(xT_sh, wT_sh):
    """Block one core's x^T and W^T shards into the kernel's DMA layouts."""
    bf = ml_dtypes.bfloat16
    # x^T k-tiles partition-major: [P, KT, T_SH], [p,k,t] = x^T[k*128+p, t]
    xp = xT_sh.reshape(KT, P, T_SH).transpose(1, 0, 2)
    # W^T per out-half: [2, P, KT, NFREE], [n,p,k,f] = W^T[k*128+p, n*512+f]
    wp = wT_sh.reshape(KT, P, 2, NFREE).transpose(2, 1, 0, 3)
    # xw packs x^T and the n=0 W tile per k: [P, KT, T_SH + NFREE]
    xw = np.ascontiguousarray(
        np.concatenate([xp, wp[0]], axis=2).astype(bf)
    )
    w1 = np.ascontiguousarray(wp[1].astype(bf))
    return {"xw": xw, "w1": w1}


def kernel(x, weight_blocks, block_rows, block_cols):
    global _last_result
    from concourse.bass_utils import run_bass_kernel_spmd

    x = np.asarray(x, dtype=np.float32)
    wT = _densify_wT(
        np.asarray(weight_blocks), np.asarray(block_rows), np.asarray(block_cols)
    )
    xT = np.ascontiguousarray(x.T)

    in_maps = []
    for c in range(N_CORES):
        tg, og = divmod(c, OG)
        in_maps.append(
            _pack_core_inputs(
                xT[:, tg * T_SH : (tg + 1) * T_SH],
                wT[:, og * O_SH : (og + 1) * O_SH],
            )
        )

    nc = _build_nc()
    res = None
    for attempt in range(3):  # transient NRT device errors happen; retry
        try:
            res = run_bass_kernel_spmd(
                nc, in_maps, core_ids=list(range(N_CORES)), trace=TRACE
            )
            break
        except Exception:
            if attempt == 2:
                raise
            import time

            time.sleep(3)
    _last_result = res

    y = np.empty((TOKENS, OUT_F), np.float32)
    for c in range(N_CORES):
        tg, og = divmod(c, OG)
        # y_d [2, P, MT, NFREE] -> [m, p] tokens x [n, f] outs
        yc = (
            np.asarray(res.results[c]["y"])
            .astype(np.float32)
            .transpose(2, 1, 0, 3)
            .reshape(T_SH, O_SH)
        )
        y[tg * T_SH : (tg + 1) * T_SH, og * O_SH : (og + 1) * O_SH] = yc
    return y


# revision 36
# speedup vs baseline: 1.0166x; 1.0166x over previous
"""Block-sparse linear y = x @ W^T on 8 Trainium2 NeuronCores.

The 32x32 block structure (50% block density, random scatter) is not
exploitable on the 128x128 PE array (matmul cost is moving-rows only, and
neither 128-aligned skip opportunities nor packable col-quadruples occur at
this density), so W^T is densified on the host (cheap scatter-add) and run
as a dense GEMM sharded 4-way over tokens x 2-way over out_features
(8 cores, no collectives).

All operands are bf16: the PE runs bf16 at the same 1 cycle/row as
float32r, but DMA traffic halves (x 4MB + W 4MB + y 2MB per core), which
puts DMA (~28us) far under the PE stream (~59us = 256 matmuls x 512 rows
@ ~2.22GHz) and makes the schedule trivially overlappable. bf16 rounding
of inputs + outputs keeps rel err ~1e-3, well inside the 2e-2 gate.

Schedule per core, two phases with no psum partials:
 (1) out-half n=0, k-outer m-inner: each k-step's x^T k-tile and W tile
     arrive PACKED as one DMA chunk (xw layout below); 8 psum banks
     accumulate the full K; drains (DVE fp32->bf16 casts + 2 batched y
     DMAs) overlap phase 2.
 (2) out-half n=1, m-outer k-inner: each bank runs its full 16-k chain
     then drains immediately; bank 7 runs as two 256-col half-chains in
     separate psum banks so only half a bank's cast + 64KB DMA trail the
     final matmul.
14 dummy 256-row warmup matmuls ramp the PE clock gate during the DMA
head so real matmuls run at ~full clock from the start (an idle gap
resets the ~3us busy-ramp, so warmups must bridge the data gate; more
warmups than the gate needs just delays the stream).

Measured HW model (from perfetto traces of this kernel):
 - each dma_start costs ~650ns of DIRECT2D descriptor generation,
   serially on its issuing sequencer -> issues are spread across the
   three DMA-capable queues (sync/scalar/gpsimd), zipped in strict
   need-order because the rings drain roughly in issue order;
 - a chunk's transfer costs ~max(1us fixed, bytes/~300GB/s shared HBM)
   regardless of partition-split tricks, so packing x+W per k-tile into
   ONE chunk (10 input dma_starts total) halves the fixed latencies and
   semaphore hops on the critical early stream;
 - fixed overheads inside the measured window: ~1.1us engine start and
   an ~8.4us NRT teardown (57 EVENT_SEMAPHORE sweeps per engine,
   PE-sequencer-bound at ~115ns each) that no kernel structure avoids;
 - PE steady state: 512-row bf16 matmul = 220ns (~2.33GHz), LDWEIGHTS
   ~97ns fully hidden; stream floor 256 x 220ns = 56.3us.
Exec ~74-78us depending on machine state (baseline fp32r kernel: ~78.6us);
run-to-run jitter is +-1.5-2us (8 cores share HBM), so schedule changes
need multi-run A/B to evaluate.
"""

import numpy as np
import ml_dtypes

TOKENS, IN_F, OUT_F = 4096, 2048, 2048
BLOCK = 32
N_CORES = 8
TG, OG = 4, 2  # token groups x out-feature groups
T_SH = TOKENS // TG  # 1024 tokens per core
O_SH = OUT_F // OG  # 1024 out features per core
P = 128
NFREE = 512  # PSUM bank free dim (fp32)
KT = IN_F // P  # 16 k tiles
MT = T_SH // P  # 8 psum banks

N_WARM = 14  # dummy 256-row matmuls to ramp the PE clock during the DMA head

MM_DTYPE = "bfloat16"  # informational; kernel is bf16-only
TRACE = False  # set by test.py to capture an NTFF profile

_nc_cache = {}
_last_result = None  # BassKernelResults of the most recent run (for test.py)


def _build_nc():
    import concourse.mybir as mybir
    import concourse.tile as tile
    from concourse import bacc

    if "nc" in _nc_cache:
        return _nc_cache["nc"]

    bf16 = mybir.dt.bfloat16
    f32 = mybir.dt.float32

    nc = bacc.Bacc(None, target_bir_lowering=False)
    # Host-pre-blocked inputs (exact SBUF layouts; all DMAs are linear):
    # xw: per k-tile, x^T columns and the phase-1 W tile PACKED side by
    #     side: [P][KT][T_SH + NFREE] (cols 0:1024 = x^T k-tile, cols
    #     1024:1536 = W^T n=0 k-tile). One dma_start per k-span delivers
    #     everything that k-step needs — per-chunk fixed latency (~1us,
    #     descriptor/completion-bound) and semaphore hops are paid once
    #     instead of twice in the critical early stream.
    # w1: phase-2 W tiles [P][KT][NFREE].
    # y:  [2][P][MT][NFREE] bf16; host reassembles tokens/outs.
    XWF = T_SH + NFREE
    xw_d = nc.dram_tensor("xw", [P, KT, XWF], bf16, kind="ExternalInput")
    w1_d = nc.dram_tensor("w1", [P, KT, NFREE], bf16, kind="ExternalInput")
    y_d = nc.dram_tensor("y", [2, P, MT, NFREE], bf16, kind="ExternalOutput")

    with tile.TileContext(nc) as tc:
        with (
            tc.tile_pool(name="xp", bufs=1) as xp,
            tc.tile_pool(name="wp", bufs=1) as wp,
            tc.tile_pool(name="op", bufs=1) as op,
            tc.tile_pool(name="ps", bufs=1, space="PSUM") as ps,
        ):
            # Warm the PE clock gate with dummy matmuls while the first DMA
            # chunks land (~2us): PE busy-time ramps the HAM clock so real
            # matmuls run at full rate almost immediately.
            zt = xp.tile([P, 256], bf16, tag="warm", name="warm")
            nc.gpsimd.memset(zt[:], 0.0)
            warm_ps = ps.tile([P, NFREE], f32, tag="ps0", name="warm_ps")
            for _ in range(N_WARM):
                nc.tensor.matmul(
                    warm_ps[:, 0:256], zt[:, 0:P], zt[:], start=True, stop=True
                )

            xw = xp.tile([P, KT, XWF], bf16, tag="xw", name="xw")
            w1t = wp.tile([P, KT, NFREE], bf16, tag="w1", name="w1")
            ot = [
                op.tile([P, MT, NFREE], bf16, tag=f"ot{n}", name=f"ot{n}")
                for n in range(2)
            ]

            def dma_xw(eng, k0, k1):
                eng.dma_start(xw[:, k0:k1, :], xw_d[:, k0:k1, :])

            def dma_w1(eng, k0, k1):
                eng.dma_start(w1t[:, k0:k1, :], w1_d[:, k0:k1, :])

            # DMA model learned from traces: each dma_start costs ~650ns of
            # DIRECT2D descriptor generation serially on its issuing
            # sequencer, and a chunk's transfer costs roughly
            # max(~1us fixed, bytes/~300GB/s shared HBM). Chunks are zipped
            # in strict need-order round-robin across the three DMA-capable
            # sequencers (the rings drain roughly in issue order, so a
            # late-need chunk issued early would starve the PE of
            # earlier-needed data). Small k-spans first for a quick gate,
            # larger spans later.
            dma_xw(nc.sync, 0, 1)       # slot 0
            dma_xw(nc.scalar, 1, 2)
            dma_xw(nc.gpsimd, 2, 4)
            dma_xw(nc.sync, 4, 6)       # slot 1
            dma_xw(nc.scalar, 6, 8)
            dma_xw(nc.gpsimd, 8, 10)
            dma_xw(nc.sync, 10, 12)     # slot 2
            dma_xw(nc.scalar, 12, 16)
            dma_w1(nc.gpsimd, 0, 8)
            dma_w1(nc.sync, 8, 16)      # slot 3

            psums = [
                ps.tile([P, NFREE], f32, tag=f"ps{m}", name=f"ps{m}")
                for m in range(MT)
            ]

            # ---- Phase 1: n=0, k-outer m-inner (matches the DMA stream);
            # full-K accumulation in 8 psum banks, no partials. ----
            for k in range(KT):
                for m in range(MT):
                    nc.tensor.matmul(
                        psums[m][:],
                        xw[:, k, m * P : (m + 1) * P],
                        xw[:, k, T_SH : T_SH + NFREE],
                        start=(k == 0),
                        stop=(k == KT - 1),
                    )
            for m in range(MT):
                nc.vector.tensor_copy(ot[0][:, m, :], psums[m][:])
            nc.scalar.dma_start(y_d[0, :, 0:4, :], ot[0][:, 0:4, :])
            nc.scalar.dma_start(y_d[0, :, 4:8, :], ot[0][:, 4:8, :])

            # ---- Phase 2: n=1, m-outer k-inner so each bank drains the
            # moment its chain finishes. Bank 7 runs as two 256-col
            # half-chains so only a half-bank cast + 64KB DMA trail the
            # final matmul. ----
            for m in range(MT):
                if m < MT - 1:
                    for k in range(KT):
                        nc.tensor.matmul(
                            psums[m][:],
                            xw[:, k, m * P : (m + 1) * P],
                            w1t[:, k, :],
                            start=(k == 0),
                            stop=(k == KT - 1),
                        )
                    nc.vector.tensor_copy(ot[1][:, m, :], psums[m][:])
                else:
                    # Two 256-col half-chains in DIFFERENT psum banks (a
                    # shared bank would serialize chain B's start=True
                    # against chain A's cast). Half A's cast+DMA overlap
                    # chain B, so only half a bank's drain trails the
                    # final matmul.
                    for h, pbank in ((0, psums[m]), (1, psums[0])):
                        c0, c1 = h * 256, (h + 1) * 256
                        for k in range(KT):
                            nc.tensor.matmul(
                                pbank[:, 0:256],
                                xw[:, k, m * P : (m + 1) * P],
                                w1t[:, k, c0:c1],
                                start=(k == 0),
                                stop=(k == KT - 1),
                            )
                        nc.vector.tensor_copy(
                            ot[1][:, m, c0:c1], pbank[:, 0:256]
                        )
                        nc.sync.dma_start(
                            y_d[1, :, m, c0:c1], ot[1][:, m, c0:c1]
                        )
                if m == 3:
                    nc.scalar.dma_start(y_d[1, :, 0:4, :], ot[1][:, 0:4, :])
                elif m == 6:
                    nc.scalar.dma_start(y_d[1, :, 4:7, :], ot[1][:, 4:7, :])

    nc.compile()
    _nc_cache["nc"] = nc
    return nc


def _densify_wT(weight_blocks, block_rows, block_cols):
    """Scatter-add the 32x32 blocks into dense W^T [in_features, out_features]."""
    nc_blk = IN_F // BLOCK
    nr_blk = OUT_F // BLOCK
    wcr = np.zeros((nc_blk, nr_blk, BLOCK, BLOCK), np.float32)
    # block b occupies W[32r:32r+32, 32c:32c+32]; W^T gets the transposed block
    np.add.at(
        wcr,
        (block_cols.astype(np.int64), block_rows.astype(np.int64)),
        np.swapaxes(weight_blocks.astype(np.float32, copy=False), 1, 2),
    )
    return np.ascontiguousarray(wcr.transpose(0, 2, 1, 3).reshape(IN_F, OUT_F))


def _pack_core_inputs(xT_sh, wT_sh):
    """Block one core's x^T and W^T shards into the kernel's DMA layouts."""
    bf = ml_dtypes.bfloat16
    # x^T k-tiles partition-major: [P, KT, T_SH], [p,k,t] = x^T[k*128+p, t]
    xp = xT_sh.reshape(KT, P, T_SH).transpose(1, 0, 2)
    # W^T per out-half: [2, P, KT, NFREE], [n,p,k,f] = W^T[k*128+p, n*512+f]
    wp = wT_sh.reshape(KT, P, 2, NFREE).transpose(2, 1, 0, 3)
    # xw packs x^T and the n=0 W tile per k: [P, KT, T_SH + NFREE]
    xw = np.ascontiguousarray(
        np.concatenate([xp, wp[0]], axis=2).astype(bf)
    )
    w1 = np.ascontiguousarray(wp[1].astype(bf))
    return {"xw": xw, "w1": w1}


def kernel(x, weight_blocks, block_rows, block_cols):
    global _last_result
    from concourse.bass_utils import run_bass_kernel_spmd

    x = np.asarray(x, dtype=np.float32)
    wT = _densify_wT(
        np.asarray(weight_blocks), np.asarray(block_rows), np.asarray(block_cols)
    )
    xT = np.ascontiguousarray(x.T)

    in_maps = []
    for c in range(N_CORES):
        tg, og = divmod(c, OG)
        in_maps.append(
            _pack_core_inputs(
                xT[:, tg * T_SH : (tg + 1) * T_SH],
                wT[:, og * O_SH : (og + 1) * O_SH],
            )
        )

    nc = _build_nc()
    res = None
    for attempt in range(3):  # transient NRT device errors happen; retry
        try:
            res = run_bass_kernel_spmd(
                nc, in_maps, core_ids=list(range(N_CORES)), trace=TRACE
            )
            break
        except Exception:
            if attempt == 2:
                raise
            import time

            time.sleep(3)
    _last_result = res

    y = np.empty((TOKENS, OUT_F), np.float32)
    for c in range(N_CORES):
        tg, og = divmod(c, OG)
        # y_d [2, P, MT, NFREE] -> [m, p] tokens x [n, f] outs
        yc = (
            np.asarray(res.results[c]["y"])
            .astype(np.float32)
            .transpose(2, 1, 0, 3)
            .reshape(T_SH, O_SH)
        )
        y[tg * T_SH : (tg + 1) * T_SH, og * O_SH : (og + 1) * O_SH] = yc
    return y


# revision 39
# speedup vs baseline: 1.0516x; 1.0345x over previous
"""Block-sparse linear y = x @ W^T on 8 Trainium2 NeuronCores.

The 32x32 block structure (50% block density, random scatter) is not
exploitable on the 128x128 PE array (matmul cost is moving-rows only, and
neither 128-aligned skip opportunities nor packable col-quadruples occur at
this density), so W^T is densified on the host (cheap scatter-add) and run
as a dense GEMM sharded 4-way over tokens x 2-way over out_features
(8 cores, no collectives).

All operands are bf16: the PE runs bf16 at the same 1 cycle/row as
float32r, but DMA traffic halves (x 4MB + W 4MB + y 2MB per core), which
puts DMA (~28us) far under the PE stream (~59us = 256 matmuls x 512 rows
@ ~2.22GHz) and makes the schedule trivially overlappable. bf16 rounding
of inputs + outputs keeps rel err ~1e-3, well inside the 2e-2 gate.

Schedule per core, two phases with no psum partials:
 (1) out-half n=0, k-outer m-inner: each k-step's x^T k-tile and W tile
     arrive PACKED as one DMA chunk (xw layout below); 8 psum banks
     accumulate the full K; drains (DVE fp32->bf16 casts + 2 batched y
     DMAs) overlap phase 2.
 (2) out-half n=1, m-outer k-inner: each bank runs its full 16-k chain
     then drains immediately; bank 7 runs as two 256-col half-chains in
     separate psum banks so only half a bank's cast + 64KB DMA trail the
     final matmul.
14 dummy 256-row warmup matmuls ramp the PE clock gate during the DMA
head so real matmuls run at ~full clock from the start (an idle gap
resets the ~3us busy-ramp, so warmups must bridge the data gate; more
warmups than the gate needs just delays the stream).

Measured HW model (from perfetto traces of this kernel):
 - each dma_start costs ~650ns of DIRECT2D descriptor generation,
   serially on its issuing sequencer; a chunk's transfer costs
   ~max(1us fixed, bytes/~300GB/s shared HBM);
 - packing x+W per k-tile into ONE chunk (11 input dma_starts total)
   makes the whole input stream fit SERIALLY on the sync queue in
   strict need-order: ring-FIFO order exactly matches consumption
   order and the k0 gate chunk transfers alone. This is worth ~4.5us
   vs spreading issues across queues (cross-queue descriptor
   interleaving delayed the gate AND caused +-2us run jitter);
 - fixed overheads inside the measured window: ~1.1us engine start and
   an ~8.4us NRT teardown (57 EVENT_SEMAPHORE sweeps per engine,
   PE-sequencer-bound at ~115ns each) that no kernel structure avoids;
 - PE steady state: 512-row bf16 matmul = 220ns (~2.33GHz), LDWEIGHTS
   ~97ns fully hidden; stream floor 256 x 220ns = 56.3us.
Exec ~71.6-72.4us (baseline fp32r kernel: ~78.6us). The serial input
stream also collapsed run-to-run jitter from +-2us to +-0.4us.
"""

import numpy as np
import ml_dtypes

TOKENS, IN_F, OUT_F = 4096, 2048, 2048
BLOCK = 32
N_CORES = 8
TG, OG = 4, 2  # token groups x out-feature groups
T_SH = TOKENS // TG  # 1024 tokens per core
O_SH = OUT_F // OG  # 1024 out features per core
P = 128
NFREE = 512  # PSUM bank free dim (fp32)
KT = IN_F // P  # 16 k tiles
MT = T_SH // P  # 8 psum banks

N_WARM = 14  # dummy 256-row matmuls to ramp the PE clock during the DMA head

MM_DTYPE = "bfloat16"  # informational; kernel is bf16-only
TRACE = False  # set by test.py to capture an NTFF profile

_nc_cache = {}
_last_result = None  # BassKernelResults of the most recent run (for test.py)


def _build_nc():
    import concourse.mybir as mybir
    import concourse.tile as tile
    from concourse import bacc

    if "nc" in _nc_cache:
        return _nc_cache["nc"]

    bf16 = mybir.dt.bfloat16
    f32 = mybir.dt.float32

    nc = bacc.Bacc(None, target_bir_lowering=False)
    # Host-pre-blocked inputs (exact SBUF layouts; all DMAs are linear):
    # xw: per k-tile, x^T columns and the phase-1 W tile PACKED side by
    #     side: [P][KT][T_SH + NFREE] (cols 0:1024 = x^T k-tile, cols
    #     1024:1536 = W^T n=0 k-tile). One dma_start per k-span delivers
    #     everything that k-step needs — per-chunk fixed latency (~1us,
    #     descriptor/completion-bound) and semaphore hops are paid once
    #     instead of twice in the critical early stream.
    # w1: phase-2 W tiles [P][KT][NFREE].
    # y:  [2][P][MT][NFREE] bf16; host reassembles tokens/outs.
    XWF = T_SH + NFREE
    xw_d = nc.dram_tensor("xw", [P, KT, XWF], bf16, kind="ExternalInput")
    w1_d = nc.dram_tensor("w1", [P, KT, NFREE], bf16, kind="ExternalInput")
    y_d = nc.dram_tensor("y", [2, P, MT, NFREE], bf16, kind="ExternalOutput")

    with tile.TileContext(nc) as tc:
        with (
            tc.tile_pool(name="xp", bufs=1) as xp,
            tc.tile_pool(name="wp", bufs=1) as wp,
            tc.tile_pool(name="op", bufs=1) as op,
            tc.tile_pool(name="ps", bufs=1, space="PSUM") as ps,
        ):
            # Warm the PE clock gate with dummy matmuls while the first DMA
            # chunks land (~2us): PE busy-time ramps the HAM clock so real
            # matmuls run at full rate almost immediately.
            zt = xp.tile([P, 256], bf16, tag="warm", name="warm")
            nc.gpsimd.memset(zt[:], 0.0)
            warm_ps = ps.tile([P, NFREE], f32, tag="ps0", name="warm_ps")
            for _ in range(N_WARM):
                nc.tensor.matmul(
                    warm_ps[:, 0:256], zt[:, 0:P], zt[:], start=True, stop=True
                )

            xw = xp.tile([P, KT, XWF], bf16, tag="xw", name="xw")
            w1t = wp.tile([P, KT, NFREE], bf16, tag="w1", name="w1")
            ot = [
                op.tile([P, MT, NFREE], bf16, tag=f"ot{n}", name=f"ot{n}")
                for n in range(2)
            ]

            def dma_xw(eng, k0, k1):
                eng.dma_start(xw[:, k0:k1, :], xw_d[:, k0:k1, :])

            def dma_w1(eng, k0, k1):
                eng.dma_start(w1t[:, k0:k1, :], w1_d[:, k0:k1, :])

            # DMA model learned from traces: each dma_start costs ~650ns of
            # DIRECT2D descriptor generation serially on its issuing
            # sequencer, and a chunk's transfer costs roughly
            # max(~1us fixed, bytes/~300GB/s shared HBM). With the packed
            # xw layout there are only 11 input chunks, so ALL of them fit
            # serially on the sync queue in strict need-order: descriptor
            # generation (~7us for all chunks) stays ahead of the byte
            # stream, the k0 gate chunk gets the rings to itself (no
            # cross-queue descriptor interleaving), and ring-FIFO order
            # exactly matches consumption order. Small k-spans first for a
            # quick gate and early margins, larger spans later.
            dma_xw(nc.sync, 0, 1)
            dma_xw(nc.sync, 1, 2)
            dma_xw(nc.sync, 2, 3)
            dma_xw(nc.sync, 3, 5)
            dma_xw(nc.sync, 5, 7)
            dma_xw(nc.sync, 7, 9)
            dma_xw(nc.sync, 9, 11)
            dma_xw(nc.sync, 11, 13)
            dma_xw(nc.sync, 13, 16)
            dma_w1(nc.sync, 0, 8)
            dma_w1(nc.sync, 8, 16)

            psums = [
                ps.tile([P, NFREE], f32, tag=f"ps{m}", name=f"ps{m}")
                for m in range(MT)
            ]

            # ---- Phase 1: n=0, k-outer m-inner (matches the DMA stream);
            # full-K accumulation in 8 psum banks, no partials. ----
            for k in range(KT):
                for m in range(MT):
                    nc.tensor.matmul(
                        psums[m][:],
                        xw[:, k, m * P : (m + 1) * P],
                        xw[:, k, T_SH : T_SH + NFREE],
                        start=(k == 0),
                        stop=(k == KT - 1),
                    )
            for m in range(MT):
                nc.vector.tensor_copy(ot[0][:, m, :], psums[m][:])
            nc.scalar.dma_start(y_d[0, :, 0:4, :], ot[0][:, 0:4, :])
            nc.scalar.dma_start(y_d[0, :, 4:8, :], ot[0][:, 4:8, :])

            # ---- Phase 2: n=1, m-outer k-inner so each bank drains the
            # moment its chain finishes. Bank 7 runs as two 256-col
            # half-chains so only a half-bank cast + 64KB DMA trail the
            # final matmul. ----
            for m in range(MT):
                if m < MT - 1:
                    for k in range(KT):
                        nc.tensor.matmul(
                            psums[m][:],
                            xw[:, k, m * P : (m + 1) * P],
                            w1t[:, k, :],
                            start=(k == 0),
                            stop=(k == KT - 1),
                        )
                    nc.vector.tensor_copy(ot[1][:, m, :], psums[m][:])
                else:
                    # Two 256-col half-chains in DIFFERENT psum banks (a
                    # shared bank would serialize chain B's start=True
                    # against chain A's cast). Half A's cast+DMA overlap
                    # chain B, so only half a bank's drain trails the
                    # final matmul.
                    for h, pbank in ((0, psums[m]), (1, psums[0])):
                        c0, c1 = h * 256, (h + 1) * 256
                        for k in range(KT):
                            nc.tensor.matmul(
                                pbank[:, 0:256],
                                xw[:, k, m * P : (m + 1) * P],
                                w1t[:, k, c0:c1],
                                start=(k == 0),
                                stop=(k == KT - 1),
                            )
                        nc.vector.tensor_copy(
                            ot[1][:, m, c0:c1], pbank[:, 0:256]
                        )
                        nc.sync.dma_start(
                            y_d[1, :, m, c0:c1], ot[1][:, m, c0:c1]
                        )
                if m == 3:
                    nc.scalar.dma_start(y_d[1, :, 0:4, :], ot[1][:, 0:4, :])
                elif m == 6:
                    nc.scalar.dma_start(y_d[1, :, 4:7, :], ot[1][:, 4:7, :])

    nc.compile()
    _nc_cache["nc"] = nc
    return nc


def _densify_wT(weight_blocks, block_rows, block_cols):
    """Scatter-add the 32x32 blocks into dense W^T [in_features, out_features]."""
    nc_blk = IN_F // BLOCK
    nr_blk = OUT_F // BLOCK
    wcr = np.zeros((nc_blk, nr_blk, BLOCK, BLOCK), np.float32)
    # block b occupies W[32r:32r+32, 32c:32c+32]; W^T gets the transposed block
    np.add.at(
        wcr,
        (block_cols.astype(np.int64), block_rows.astype(np.int64)),
        np.swapaxes(weight_blocks.astype(np.float32, copy=False), 1, 2),
    )
    return np.ascontiguousarray(wcr.transpose(0, 2, 1, 3).reshape(IN_F, OUT_F))


def _pack_core_inputs(xT_sh, wT_sh):
    """Block one core's x^T and W^T shards into the kernel's DMA layouts."""
    bf = ml_dtypes.bfloat16
    # x^T k-tiles partition-major: [P, KT, T_SH], [p,k,t] = x^T[k*128+p, t]
    xp = xT_sh.reshape(KT, P, T_SH).transpose(1, 0, 2)
    # W^T per out-half: [2, P, KT, NFREE], [n,p,k,f] = W^T[k*128+p, n*512+f]
    wp = wT_sh.reshape(KT, P, 2, NFREE).transpose(2, 1, 0, 3)
    # xw packs x^T and the n=0 W tile per k: [P, KT, T_SH + NFREE]
    xw = np.ascontiguousarray(
        np.concatenate([xp, wp[0]], axis=2).astype(bf)
    )
    w1 = np.ascontiguousarray(wp[1].astype(bf))
    return {"xw": xw, "w1": w1}


def kernel(x, weight_blocks, block_rows, block_cols):
    global _last_result
    from concourse.bass_utils import run_bass_kernel_spmd

    x = np.asarray(x, dtype=np.float32)
    wT = _densify_wT(
        np.asarray(weight_blocks), np.asarray(block_rows), np.asarray(block_cols)
    )
    xT = np.ascontiguousarray(x.T)

    in_maps = []
    for c in range(N_CORES):
        tg, og = divmod(c, OG)
        in_maps.append(
            _pack_core_inputs(
                xT[:, tg * T_SH : (tg + 1) * T_SH],
                wT[:, og * O_SH : (og + 1) * O_SH],
            )
        )

    nc = _build_nc()
    res = None
    for attempt in range(3):  # transient NRT device errors happen; retry
        try:
            res = run_bass_kernel_spmd(
                nc, in_maps, core_ids=list(range(N_CORES)), trace=TRACE
            )
            break
        except Exception:
            if attempt == 2:
                raise
            import time

            time.sleep(3)
    _last_result = res

    y = np.empty((TOKENS, OUT_F), np.float32)
    for c in range(N_CORES):
        tg, og = divmod(c, OG)
        # y_d [2, P, MT, NFREE] -> [m, p] tokens x [n, f] outs
        yc = (
            np.asarray(res.results[c]["y"])
            .astype(np.float32)
            .transpose(2, 1, 0, 3)
            .reshape(T_SH, O_SH)
        )
        y[tg * T_SH : (tg + 1) * T_SH, og * O_SH : (og + 1) * O_SH] = yc
    return y
